# revision 3
# baseline (speedup 1.0000x reference)
"""Trainium2 Bass kernel for nn_NodeNet (GNN message passing).

Strategy: data-parallel over graphs across 8 NeuronCores. Host transposes
inputs into [feature, row] layouts so every DMA is contiguous; all matmuls
run in bf16 with transposed activations:
  node stage: dp[128, rows] -> MLP -> sum over datapoints -> feature_enc[64, G]
  edge stage: rhs = [fe (broadcast per graph); edge_attr^T] -> MLP -> out^T
The structured fast path (edges grouped 128-per-graph, all within-graph, as
produced by the reference's setup_inputs) runs one fused launch per core with
feature_enc kept on-chip, an interleaved chunk schedule (node chunk c+1 is
emitted before edge chunk c so the PE never drains), elementwise work split
across ACT/DVE/Pool, and the final bias folded into host post-processing so
the edge output streams psum->bf16->DRAM with a single Pool copy.
A general fallback handles arbitrary edge_index / batch with two launches and
a host-side gather + mask.
"""

import os
import sys

import ml_dtypes
import numpy as np

BF16NP = ml_dtypes.bfloat16

if "/opt/trn_rl_repo" not in sys.path and os.path.isdir("/opt/trn_rl_repo"):
    sys.path.insert(0, "/opt/trn_rl_repo")

import concourse.bacc as bacc
import concourse.tile as tile
from concourse import mybir
from concourse.bass_utils import run_bass_kernel_spmd

G, ODE, NDATA, H, EA, EPG = 4096, 64, 32, 256, 64, 128
E = G * EPG
NCORES = 8
GC = G // NCORES           # graphs per core
RC = GC * NDATA            # node-MLP rows per core
EC = GC * EPG              # edges per core
TN = 512                   # tile free size
CH = 64                    # graphs per interleave chunk
NCH = GC // CH             # chunks per core (8)
NNI = CH * NDATA // TN     # node iters per chunk (4)
NEI = CH * EPG // TN       # edge iters per chunk (16)
GPEI = TN // EPG           # graphs per edge iter (4)
GPNI = TN // NDATA         # graphs per node iter (16)

F32 = mybir.dt.float32
BF16 = mybir.dt.bfloat16
RELU = mybir.ActivationFunctionType.Relu
IDENT = mybir.ActivationFunctionType.Identity
ADD = mybir.AluOpType.add
MAX = mybir.AluOpType.max
AXX = mybir.AxisListType.X

_PROGRAMS = {}
last_results = None


def _install_trace_shim():
    """Optional: make trace=True work by injecting antenv.axon_hooks."""
    import types

    if "antenv.axon_hooks" in sys.modules:
        return
    try:
        mod = types.ModuleType("antenv.axon_hooks")
        mod._hook = None
        mod.set_axon_ntff_profile_hook = lambda h: setattr(mod, "_hook", h)
        mod.get_axon_ntff_profile_hook = lambda: mod._hook
        sys.modules["antenv.axon_hooks"] = mod
        import antenv

        antenv.axon_hooks = mod
        from trn_agent_boot.trn_boot import _ntff_profile_via_ctypes

        hook = _ntff_profile_via_ctypes("/opt/axon/libaxon_pjrt.so")
        if hook is not None:
            mod.set_axon_ntff_profile_hook(hook)
    except Exception:
        pass


def _declare_weights(nc, with_eb3=True):
    t = {}
    t["nw1"] = nc.dram_tensor("nw1", [128, H], BF16, kind="ExternalInput")
    t["nw2"] = nc.dram_tensor("nw2", [128, 2, H], BF16, kind="ExternalInput")
    t["nw3"] = nc.dram_tensor("nw3", [128, 2, ODE], BF16, kind="ExternalInput")
    t["nb1"] = nc.dram_tensor("nb1", [128, 2], F32, kind="ExternalInput")
    t["nb2"] = nc.dram_tensor("nb2", [128, 2], F32, kind="ExternalInput")
    t["nb3"] = nc.dram_tensor("nb3", [ODE, 1], F32, kind="ExternalInput")
    t["ew1"] = nc.dram_tensor("ew1", [128, H], BF16, kind="ExternalInput")
    t["ew2"] = nc.dram_tensor("ew2", [128, 2, H], BF16, kind="ExternalInput")
    t["ew3"] = nc.dram_tensor("ew3", [128, 2, ODE], BF16, kind="ExternalInput")
    t["eb1"] = nc.dram_tensor("eb1", [128, 2], F32, kind="ExternalInput")
    t["eb2"] = nc.dram_tensor("eb2", [128, 2], F32, kind="ExternalInput")
    if with_eb3:
        t["eb3"] = nc.dram_tensor("eb3", [EA, 1], F32, kind="ExternalInput")
    return t


def _load_weights(nc, consts, td, node: bool, edge: bool, with_eb3=True):
    sb = {}
    names = []
    if node:
        names += ["nw1", "nw2", "nw3", "nb1", "nb2", "nb3"]
    if edge:
        names += ["ew1", "ew2", "ew3", "eb1", "eb2"]
        if with_eb3:
            names += ["eb3"]
    for n in names:
        d = td[n]
        sb[n] = consts.tile(list(d.shape), d.dtype, tag=n, name=n)
        nc.sync.dma_start(sb[n], d[:])
    return sb


def _build_fused2():
    """Structured fast path: interleaved node/edge chunks, bf16 output,
    edge bias eb3 added on host.

    Iterations are processed in pairs sharing one L1 psum tile so the h1
    bias+relu runs as a single 1024-col instruction per engine. The PE
    emission order is software-pipelined (next pair's L1 matmuls sit
    between L3(i) and L2(i+1)) so the single-buffered L2 psum never
    stalls the PE. GPSIMD cannot touch PSUM, so it only gets SBUF work:
    the fe broadcast and the first half of the node datapoint reduction.
    """
    nc = bacc.Bacc("TRN2", target_bir_lowering=False)
    td = _declare_weights(nc, with_eb3=False)
    xT_d = nc.dram_tensor("xT", [128, RC], BF16, kind="ExternalInput")
    attrT_d = nc.dram_tensor("attrT", [64, EC], BF16, kind="ExternalInput")
    outT_d = nc.dram_tensor("outT", [64, EC], BF16, kind="ExternalOutput")

    with tile.TileContext(nc) as tc:
        with (
            tc.tile_pool(name="consts", bufs=1) as consts,
            tc.tile_pool(name="xin", bufs=6) as xin,
            tc.tile_pool(name="hid", bufs=2) as hid,
            tc.tile_pool(name="oot", bufs=4) as oot,
            tc.tile_pool(name="hsp", bufs=2) as hsp,
            tc.tile_pool(name="ps1", bufs=1, space="PSUM") as ps1,
            tc.tile_pool(name="ps2", bufs=1, space="PSUM") as ps2,
            tc.tile_pool(name="ps3", bufs=2, space="PSUM") as ps3,
        ):
            w = _load_weights(nc, consts, td, node=True, edge=True, with_eb3=False)
            feT = consts.tile([ODE, GC], BF16, tag="feT")

            # Streams of per-pair work items. Each item is a dict of
            # closures emitted in pipelined order:
            #   rt(pair)   - input tiles ready (dma/bcast), returns tiles
            #   l1(pair)   - L1 matmuls into the shared l1 psum + h1 acts
            #   body(iter) - L2 + h2 acts + L3 + out for one iteration
            #   tail()     - end-of-chunk work (fe matmul) or None

            def node_pairs(c):
                hsum = hsp.tile([128, 2, CH], BF16, tag="hsum", name="hsum")
                for p in range(NNI // 2):
                    def mk_rt(c=c, p=p):
                        xts = []
                        for j in (0, 1):
                            col0 = c * CH * NDATA + (2 * p + j) * TN
                            xt = xin.tile([128, TN], BF16, tag="xt", name="xt")
                            nc.sync.dma_start(xt, xT_d[:, col0:col0 + TN])
                            xts.append(xt)
                        return xts

                    def mk_l1(xts):
                        l1 = ps1.tile([128, 2, 2, TN], F32, tag="l1", name="l1")
                        h1 = hid.tile([128, 2, 2, TN], BF16, tag="h1", name="h1")
                        for j in (0, 1):
                            nc.tensor.matmul(l1[:, 0, j], w["nw1"][:, 0:128],
                                             xts[j], start=True, stop=True)
                            nc.tensor.matmul(l1[:, 1, j], w["nw1"][:, 128:256],
                                             xts[j], start=True, stop=True)
                        nc.scalar.activation(h1[:, 0], l1[:, 0], RELU,
                                             bias=w["nb1"][:, 0:1])
                        nc.vector.tensor_scalar(
                            out=h1[:, 1], in0=l1[:, 1], scalar1=w["nb1"][:, 1:2],
                            scalar2=0.0, op0=ADD, op1=MAX)
                        return h1

                    def mk_body(h1, j, c=c, p=p):
                        i = 2 * p + j
                        l2 = ps2.tile([128, 2, TN], F32, tag="l2", name="l2")
                        for m in (0, 1):
                            for k in (0, 1):
                                nc.tensor.matmul(
                                    l2[:, m], w["nw2"][:, k, m * 128:(m + 1) * 128],
                                    h1[:, k, j], start=(k == 0), stop=(k == 1))
                        h2 = hid.tile([128, 2, TN], BF16, tag="h2", name="h2")
                        nc.scalar.activation(h2[:, 0], l2[:, 0], RELU,
                                             bias=w["nb2"][:, 0:1])
                        if i % 2 == 0:
                            nc.scalar.activation(h2[:, 1], l2[:, 1], RELU,
                                                 bias=w["nb2"][:, 1:2])
                        else:
                            nc.vector.tensor_scalar(
                                out=h2[:, 1], in0=l2[:, 1],
                                scalar1=w["nb2"][:, 1:2],
                                scalar2=0.0, op0=ADD, op1=MAX)
                        # datapoint reduction, stage 1 on GPSIMD (SBUF only):
                        # fold the two 16-datapoint halves, then DVE reduces 16.
                        h2r = h2.rearrange("c k (g d) -> c k g d", d=NDATA)
                        part = hid.tile([128, 2, GPNI, NDATA // 2], BF16,
                                        tag="part", name="part")
                        with nc.allow_low_precision(reason="bf16 partial sums"):
                            nc.gpsimd.tensor_tensor(
                                out=part, in0=h2r[:, :, :, 0:NDATA // 2],
                                in1=h2r[:, :, :, NDATA // 2:NDATA], op=ADD)
                            nc.vector.reduce_sum(
                                out=hsum[:, :, i * GPNI:(i + 1) * GPNI],
                                in_=part.rearrange("c k g d -> c (k g) d"),
                                axis=AXX)

                    def mk_tail(c=c, p=p):
                        if p != NNI // 2 - 1:
                            return
                        l3 = ps3.tile([ODE, CH], F32, tag="l3", name="l3f")
                        for k in (0, 1):
                            nc.tensor.matmul(l3, w["nw3"][:, k], hsum[:, k],
                                             start=(k == 0), stop=(k == 1))
                        nc.scalar.activation(feT[:, c * CH:(c + 1) * CH], l3,
                                             IDENT, bias=w["nb3"])

                    yield dict(rt=mk_rt, l1=mk_l1, body=mk_body, tail=mk_tail)

            def edge_pairs(c):
                for p in range(NEI // 2):
                    def mk_rt(c=c, p=p):
                        rts = []
                        for j in (0, 1):
                            i = 2 * p + j
                            e0 = c * CH * EPG + i * TN
                            g0 = c * CH + i * GPEI
                            rt = xin.tile([128, TN], BF16, tag="rt", name="rt")
                            nc.sync.dma_start(rt[64:128], attrT_d[:, e0:e0 + TN])
                            nc.gpsimd.tensor_copy(
                                out=rt[0:64].rearrange("c (g e) -> c g e", e=EPG),
                                in_=feT[:, g0:g0 + GPEI, None].to_broadcast(
                                    [ODE, GPEI, EPG]))
                            rts.append(rt)
                        return rts

                    def mk_l1(rts):
                        l1 = ps1.tile([128, 2, 2, TN], F32, tag="l1", name="l1")
                        h1 = hid.tile([128, 2, 2, TN], BF16, tag="h1", name="h1")
                        for j in (0, 1):
                            nc.tensor.matmul(l1[:, 0, j], w["ew1"][:, 0:128],
                                             rts[j], start=True, stop=True)
                            nc.tensor.matmul(l1[:, 1, j], w["ew1"][:, 128:256],
                                             rts[j], start=True, stop=True)
                        nc.scalar.activation(h1[:, 0], l1[:, 0], RELU,
                                             bias=w["eb1"][:, 0:1])
                        nc.vector.tensor_scalar(
                            out=h1[:, 1], in0=l1[:, 1], scalar1=w["eb1"][:, 1:2],
                            scalar2=0.0, op0=ADD, op1=MAX)
                        return h1

                    def mk_body(h1, j, c=c, p=p):
                        i = 2 * p + j
                        e0 = c * CH * EPG + i * TN
                        l2 = ps2.tile([128, 2, TN], F32, tag="l2", name="l2")
                        for m in (0, 1):
                            for k in (0, 1):
                                nc.tensor.matmul(
                                    l2[:, m], w["ew2"][:, k, m * 128:(m + 1) * 128],
                                    h1[:, k, j], start=(k == 0), stop=(k == 1))
                        h2 = hid.tile([128, 2, TN], BF16, tag="h2", name="h2")
                        nc.scalar.activation(h2[:, 0], l2[:, 0], RELU,
                                             bias=w["eb2"][:, 0:1])
                        nc.vector.tensor_scalar(
                            out=h2[:, 1], in0=l2[:, 1], scalar1=w["eb2"][:, 1:2],
                            scalar2=0.0, op0=ADD, op1=MAX)
                        l3 = ps3.tile([ODE, TN], F32, tag="l3", name="l3")
                        for k in (0, 1):
                            nc.tensor.matmul(l3, w["ew3"][:, k], h2[:, k],
                                             start=(k == 0), stop=(k == 1))
                        ot = oot.tile([ODE, TN], BF16, tag="ot", name="ot")
                        with nc.allow_low_precision(reason="bf16 out, bias on host"):
                            if i % 2 == 0:
                                nc.scalar.activation(
                                    ot, l3, mybir.ActivationFunctionType.Copy)
                            else:
                                nc.vector.tensor_copy(out=ot, in_=l3)
                        nc.sync.dma_start(outT_d[:, e0:e0 + TN], ot)

                    yield dict(rt=mk_rt, l1=mk_l1, body=mk_body, tail=None)

            def run_pipelined(pairs):
                """Emit pair stream software-pipelined: the L1 matmuls of
                pair p+1 are emitted between body(2p) and body(2p+1)."""
                pairs = list(pairs)
                # prologue: inputs for pair 0 and 1, L1 of pair 0
                if not pairs:
                    return
                rt_cur = pairs[0]["rt"]()
                h1_cur = pairs[0]["l1"](rt_cur)
                rt_next = pairs[1]["rt"]() if len(pairs) > 1 else None
                for p, pr in enumerate(pairs):
                    pr["body"](h1_cur, 0)
                    if p + 1 < len(pairs):
                        h1_next = pairs[p + 1]["l1"](rt_next)
                        rt_next = pairs[p + 2]["rt"]() if p + 2 < len(pairs) else None
                    else:
                        h1_next = None
                    pr["body"](h1_cur, 1)
                    if pr["tail"] is not None:
                        pr["tail"]()
                    h1_cur = h1_next

            streams = [list(node_pairs(0))]
            for c in range(NCH):
                if c + 1 < NCH:
                    streams.append(list(node_pairs(c + 1)))
                streams.append(list(edge_pairs(c)))
            run_pipelined([pr for st in streams for pr in st])
    nc.finalize()
    return nc


# ---------------- general fallback (arbitrary edge_index/batch) ----------------

def _emit_node_stage(nc, pools, w, xT_d, hsum):
    consts, xin, hid, ps1, ps2, ps3 = pools
    GT = TN // NDATA
    for p in range(RC // (2 * TN)):
        r0 = p * 2 * TN
        xtp = xin.tile([128, 2, TN], BF16, tag="xt")
        nc.sync.dma_start(xtp, xT_d[:, r0:r0 + 2 * TN].rearrange("c (t e) -> c t e", t=2))
        h1p = hid.tile([128, 2, 2, TN], BF16, tag="h1")
        for t01 in (0, 1):
            ps_a = ps1.tile([128, TN], F32, tag="l1a")
            ps_b = ps1.tile([128, TN], F32, tag="l1b")
            nc.tensor.matmul(ps_a, w["nw1"][:, 0:128], xtp[:, t01], start=True, stop=True)
            nc.tensor.matmul(ps_b, w["nw1"][:, 128:256], xtp[:, t01], start=True, stop=True)
            nc.scalar.activation(h1p[:, 0, t01], ps_a, RELU, bias=w["nb1"][:, 0:1])
            nc.vector.tensor_scalar(
                out=h1p[:, 1, t01], in0=ps_b, scalar1=w["nb1"][:, 1:2], scalar2=0.0,
                op0=ADD, op1=MAX,
            )
        l2ap = ps2.tile([128, 2, TN], F32, tag="l2a")
        l2bp = ps2.tile([128, 2, TN], F32, tag="l2b")
        for t01 in (0, 1):
            for k in (0, 1):
                nc.tensor.matmul(l2ap[:, t01], w["nw2"][:, k, 0:128], h1p[:, k, t01],
                                 start=(k == 0), stop=(k == 1))
            for k in (0, 1):
                nc.tensor.matmul(l2bp[:, t01], w["nw2"][:, k, 128:256], h1p[:, k, t01],
                                 start=(k == 0), stop=(k == 1))
        h2p = hid.tile([128, 2, 2, TN], BF16, tag="h2")
        nc.scalar.activation(h2p[:, 0], l2ap, RELU, bias=w["nb2"][:, 0:1])
        nc.vector.tensor_scalar(
            out=h2p[:, 1], in0=l2bp, scalar1=w["nb2"][:, 1:2], scalar2=0.0,
            op0=ADD, op1=MAX,
        )
        with nc.allow_low_precision(reason="bf16 reduce feeds bf16 matmul"):
            nc.vector.reduce_sum(
                out=hsum[:, :, p * 2 * GT:(p + 1) * 2 * GT],
                in_=h2p.rearrange("c k t (g d) -> c (k t g) d", d=NDATA),
                axis=AXX,
            )
    ps_f = ps3.tile([ODE, 2, TN], F32, tag="l3")
    for k in (0, 1):
        nc.tensor.matmul(ps_f[:, 0], w["nw3"][:, k], hsum[:, k],
                         start=(k == 0), stop=(k == 1))
    return ps_f[:, 0]


def _emit_edge_stage(nc, pools, w, attrT_d, outT_d, feTg_d):
    consts, xin, hid, ps1, ps2, ps3 = pools
    for p in range(EC // (2 * TN)):
        e0 = p * 2 * TN
        rtp = xin.tile([128, 2, TN], BF16, tag="rt")
        nc.sync.dma_start(rtp[64:128],
                          attrT_d[:, e0:e0 + 2 * TN].rearrange("c (t e) -> c t e", t=2))
        nc.sync.dma_start(rtp[0:64],
                          feTg_d[:, e0:e0 + 2 * TN].rearrange("c (t e) -> c t e", t=2))
        e1p = hid.tile([128, 2, 2, TN], BF16, tag="h1")
        for t01 in (0, 1):
            ps_a = ps1.tile([128, TN], F32, tag="l1a")
            ps_b = ps1.tile([128, TN], F32, tag="l1b")
            nc.tensor.matmul(ps_a, w["ew1"][:, 0:128], rtp[:, t01], start=True, stop=True)
            nc.tensor.matmul(ps_b, w["ew1"][:, 128:256], rtp[:, t01], start=True, stop=True)
            nc.scalar.activation(e1p[:, 0, t01], ps_a, RELU, bias=w["eb1"][:, 0:1])
            nc.vector.tensor_scalar(
                out=e1p[:, 1, t01], in0=ps_b, scalar1=w["eb1"][:, 1:2], scalar2=0.0,
                op0=ADD, op1=MAX,
            )
        l2ap = ps2.tile([128, 2, TN], F32, tag="l2a")
        l2bp = ps2.tile([128, 2, TN], F32, tag="l2b")
        for t01 in (0, 1):
            for k in (0, 1):
                nc.tensor.matmul(l2ap[:, t01], w["ew2"][:, k, 0:128], e1p[:, k, t01],
                                 start=(k == 0), stop=(k == 1))
            for k in (0, 1):
                nc.tensor.matmul(l2bp[:, t01], w["ew2"][:, k, 128:256], e1p[:, k, t01],
                                 start=(k == 0), stop=(k == 1))
        e2p = hid.tile([128, 2, 2, TN], BF16, tag="h2")
        nc.scalar.activation(e2p[:, 0], l2ap, RELU, bias=w["eb2"][:, 0:1])
        nc.vector.tensor_scalar(
            out=e2p[:, 1], in0=l2bp, scalar1=w["eb2"][:, 1:2], scalar2=0.0,
            op0=ADD, op1=MAX,
        )
        l3p = ps3.tile([ODE, 2, TN], F32, tag="l3")
        for t01 in (0, 1):
            for k in (0, 1):
                nc.tensor.matmul(l3p[:, t01], w["ew3"][:, k], e2p[:, k, t01],
                                 start=(k == 0), stop=(k == 1))
        otp = hid.tile([ODE, 2, TN], F32, tag="ot")
        if p % 2 == 0:
            nc.scalar.activation(otp, l3p, IDENT, bias=w["eb3"])
        else:
            nc.vector.tensor_scalar(out=otp, in0=l3p, scalar1=w["eb3"],
                                    scalar2=0.0, op0=ADD, op1=mybir.AluOpType.bypass)
        nc.sync.dma_start(outT_d[:, e0:e0 + 2 * TN],
                          otp.rearrange("c t e -> c (t e)"))


def _build_general(mode):
    """mode: 'node' or 'edge' (general fallback path)."""
    nc = bacc.Bacc("TRN2", target_bir_lowering=False)
    td = _declare_weights(nc)
    if mode == "node":
        xT_d = nc.dram_tensor("xT", [128, RC], BF16, kind="ExternalInput")
        feT_out = nc.dram_tensor("feT", [ODE, GC], F32, kind="ExternalOutput")
    else:
        attrT_d = nc.dram_tensor("attrT", [64, EC], BF16, kind="ExternalInput")
        outT_d = nc.dram_tensor("outT", [64, EC], F32, kind="ExternalOutput")
        feTg_d = nc.dram_tensor("feTg", [64, EC], BF16, kind="ExternalInput")

    with tile.TileContext(nc) as tc:
        with (
            tc.tile_pool(name="consts", bufs=1) as consts,
            tc.tile_pool(name="xin", bufs=4) as xin,
            tc.tile_pool(name="hid", bufs=3) as hid,
            tc.tile_pool(name="ps1", bufs=1, space="PSUM") as ps1,
            tc.tile_pool(name="ps2", bufs=1, space="PSUM") as ps2,
            tc.tile_pool(name="ps3", bufs=1, space="PSUM") as ps3,
        ):
            pools = (consts, xin, hid, ps1, ps2, ps3)
            w = _load_weights(nc, consts, td, node=mode == "node", edge=mode == "edge")
            if mode == "node":
                hsum = consts.tile([128, 2, GC], BF16, tag="hsum")
                ps_f = _emit_node_stage(nc, pools, w, xT_d, hsum)
                feT_sb = consts.tile([ODE, GC], F32, tag="feT")
                nc.scalar.activation(feT_sb, ps_f, IDENT, bias=w["nb3"])
                nc.sync.dma_start(feT_out[:], feT_sb)
            else:
                _emit_edge_stage(nc, pools, w, attrT_d, outT_d, feTg_d)
    nc.finalize()
    return nc


def _get_program(mode):
    if mode not in _PROGRAMS:
        if mode == "fused2":
            _PROGRAMS[mode] = _build_fused2()
        else:
            _PROGRAMS[mode] = _build_general(mode)
    return _PROGRAMS[mode]


def _shared_weight_arrays(kw):
    f = np.float32
    c = np.ascontiguousarray
    return {
        "nw1": c(np.asarray(kw["node_w1"], dtype=f).astype(BF16NP)),
        "nw2": c(np.asarray(kw["node_w2"], dtype=f).reshape(2, 128, H).transpose(1, 0, 2).astype(BF16NP)),
        "nw3": c(np.asarray(kw["node_w3"], dtype=f).reshape(2, 128, ODE).transpose(1, 0, 2).astype(BF16NP)),
        "nb1": c(np.asarray(kw["node_b1"], dtype=f).reshape(2, 128).T),
        "nb2": c(np.asarray(kw["node_b2"], dtype=f).reshape(2, 128).T),
        "nb3": c(np.asarray(kw["node_b3"], dtype=f).reshape(ODE, 1)),
        "ew1": c(np.asarray(kw["edge_w1"], dtype=f).astype(BF16NP)),
        "ew2": c(np.asarray(kw["edge_w2"], dtype=f).reshape(2, 128, H).transpose(1, 0, 2).astype(BF16NP)),
        "ew3": c(np.asarray(kw["edge_w3"], dtype=f).reshape(2, 128, ODE).transpose(1, 0, 2).astype(BF16NP)),
        "eb1": c(np.asarray(kw["edge_b1"], dtype=f).reshape(2, 128).T),
        "eb2": c(np.asarray(kw["edge_b2"], dtype=f).reshape(2, 128).T),
        "eb3": c(np.asarray(kw["edge_b3"], dtype=f).reshape(EA, 1)),
    }


def _x_transposed_per_core(x, c):
    xs = np.asarray(x, dtype=np.float32).reshape(G, ODE, 2, NDATA)[c * GC:(c + 1) * GC]
    return np.ascontiguousarray(xs.transpose(1, 2, 0, 3).reshape(128, RC).astype(BF16NP))


def kernel(x, edge_attr, node_w1, node_b1, node_w2, node_b2, node_w3, node_b3,
           edge_w1, edge_b1, edge_w2, edge_b2, edge_w3, edge_b3,
           edge_index, batch):
    global last_results
    kw = dict(x=x, node_w1=node_w1, node_b1=node_b1, node_w2=node_w2,
              node_b2=node_b2, node_w3=node_w3, node_b3=node_b3,
              edge_w1=edge_w1, edge_b1=edge_b1, edge_w2=edge_w2,
              edge_b2=edge_b2, edge_w3=edge_w3, edge_b3=edge_b3)
    trace = os.environ.get("KERNEL_TRACE", "") == "1"
    if trace:
        _install_trace_shim()

    edge_attr = np.asarray(edge_attr, dtype=np.float32)
    ei = np.asarray(edge_index)
    bt = np.asarray(batch)
    g_src = bt[ei[0]]
    g_dst = bt[ei[1]]
    same = g_src == g_dst
    structured = bool((g_src == np.repeat(np.arange(G), EPG)).all())

    shared = _shared_weight_arrays(kw)
    run_kwargs = dict(core_ids=list(range(NCORES)), trace=trace,
                      trace_cores=[0] if trace else None)

    if structured:
        nc = _get_program("fused2")
        eb3_host = np.asarray(kw["edge_b3"], dtype=np.float32).reshape(1, EA)
        in_maps = []
        for c in range(NCORES):
            m = {k: v for k, v in shared.items() if k != "eb3"}
            m["xT"] = _x_transposed_per_core(x, c)
            m["attrT"] = np.ascontiguousarray(edge_attr[c * EC:(c + 1) * EC].T.astype(BF16NP))
            in_maps.append(m)
        res = run_bass_kernel_spmd(nc, in_maps, **run_kwargs)
        last_results = res
        out = np.empty((E, EA), dtype=np.float32)
        for c in range(NCORES):
            out[c * EC:(c + 1) * EC] = res.results[c]["outT"].T.astype(np.float32)
        out += eb3_host
    else:
        # general path: node stage -> host gather of feature_enc -> edge stage
        nc_node = _get_program("node")
        in_maps = []
        for c in range(NCORES):
            m = dict(shared)
            m["xT"] = _x_transposed_per_core(x, c)
            in_maps.append(m)
        res_n = run_bass_kernel_spmd(nc_node, in_maps, **run_kwargs)
        feT_full = np.concatenate([res_n.results[c]["feT"] for c in range(NCORES)],
                                  axis=1)          # [64, G]
        feTg = feT_full[:, g_src]                   # [64, E]
        nc_edge = _get_program("edge")
        in_maps = []
        for c in range(NCORES):
            m = dict(shared)
            m["attrT"] = np.ascontiguousarray(edge_attr[c * EC:(c + 1) * EC].T.astype(BF16NP))
            m["feTg"] = np.ascontiguousarray(feTg[:, c * EC:(c + 1) * EC].astype(BF16NP))
            in_maps.append(m)
        res = run_bass_kernel_spmd(nc_edge, in_maps, **run_kwargs)
        last_results = res
        out = np.empty((E, EA), dtype=np.float32)
        for c in range(NCORES):
            out[c * EC:(c + 1) * EC] = res.results[c]["outT"].T
    if not same.all():
        out = np.where(same[:, None], out, edge_attr)
    return out


# revision 5
# speedup vs baseline: 1.0133x; 1.0133x over previous
"""Trainium2 Bass kernel for nn_NodeNet (GNN message passing).

Strategy: data-parallel over graphs across 8 NeuronCores. Host transposes
inputs into [feature, row] layouts so every DMA is contiguous; all matmuls
run in bf16 with transposed activations:
  node stage: dp[128, rows] -> MLP -> sum over datapoints -> feature_enc[64, G]
  edge stage: rhs = [fe (broadcast per graph); edge_attr^T] -> MLP -> out^T
The structured fast path (edges grouped 128-per-graph, all within-graph, as
produced by the reference's setup_inputs) runs one fused launch per core with
feature_enc kept on-chip, an interleaved chunk schedule (node chunk c+1 is
emitted before edge chunk c so the PE never drains), elementwise work split
across ACT/DVE/Pool, and the final bias folded into host post-processing so
the edge output streams psum->bf16->DRAM with a single Pool copy.
A general fallback handles arbitrary edge_index / batch with two launches and
a host-side gather + mask.
"""

import os
import sys

import ml_dtypes
import numpy as np

BF16NP = ml_dtypes.bfloat16

if "/opt/trn_rl_repo" not in sys.path and os.path.isdir("/opt/trn_rl_repo"):
    sys.path.insert(0, "/opt/trn_rl_repo")

import concourse.bacc as bacc
import concourse.tile as tile
from concourse import mybir
from concourse.bass_utils import run_bass_kernel_spmd

G, ODE, NDATA, H, EA, EPG = 4096, 64, 32, 256, 64, 128
E = G * EPG
NCORES = 8
GC = G // NCORES           # graphs per core
RC = GC * NDATA            # node-MLP rows per core
EC = GC * EPG              # edges per core
TN = 512                   # tile free size
CH = 64                    # graphs per interleave chunk
NCH = GC // CH             # chunks per core (8)
NNI = CH * NDATA // TN     # node iters per chunk (4)
NEI = CH * EPG // TN       # edge iters per chunk (16)
GPEI = TN // EPG           # graphs per edge iter (4)
GPNI = TN // NDATA         # graphs per node iter (16)

F32 = mybir.dt.float32
BF16 = mybir.dt.bfloat16
RELU = mybir.ActivationFunctionType.Relu
IDENT = mybir.ActivationFunctionType.Identity
ADD = mybir.AluOpType.add
MAX = mybir.AluOpType.max
AXX = mybir.AxisListType.X

_PROGRAMS = {}
last_results = None


def _install_trace_shim():
    """Optional: make trace=True work by injecting antenv.axon_hooks."""
    import types

    if "antenv.axon_hooks" in sys.modules:
        return
    try:
        mod = types.ModuleType("antenv.axon_hooks")
        mod._hook = None
        mod.set_axon_ntff_profile_hook = lambda h: setattr(mod, "_hook", h)
        mod.get_axon_ntff_profile_hook = lambda: mod._hook
        sys.modules["antenv.axon_hooks"] = mod
        import antenv

        antenv.axon_hooks = mod
        from trn_agent_boot.trn_boot import _ntff_profile_via_ctypes

        hook = _ntff_profile_via_ctypes("/opt/axon/libaxon_pjrt.so")
        if hook is not None:
            mod.set_axon_ntff_profile_hook(hook)
    except Exception:
        pass


def _declare_weights(nc, with_eb3=True):
    t = {}
    t["nw1"] = nc.dram_tensor("nw1", [128, H], BF16, kind="ExternalInput")
    t["nw2"] = nc.dram_tensor("nw2", [128, 2, H], BF16, kind="ExternalInput")
    t["nw3"] = nc.dram_tensor("nw3", [128, 2, ODE], BF16, kind="ExternalInput")
    t["nb1"] = nc.dram_tensor("nb1", [128, 2], F32, kind="ExternalInput")
    t["nb2"] = nc.dram_tensor("nb2", [128, 2], F32, kind="ExternalInput")
    t["nb3"] = nc.dram_tensor("nb3", [ODE, 1], F32, kind="ExternalInput")
    t["ew1"] = nc.dram_tensor("ew1", [128, H], BF16, kind="ExternalInput")
    t["ew2"] = nc.dram_tensor("ew2", [128, 2, H], BF16, kind="ExternalInput")
    t["ew3"] = nc.dram_tensor("ew3", [128, 2, ODE], BF16, kind="ExternalInput")
    t["eb1"] = nc.dram_tensor("eb1", [128, 2], F32, kind="ExternalInput")
    t["eb2"] = nc.dram_tensor("eb2", [128, 2], F32, kind="ExternalInput")
    if with_eb3:
        t["eb3"] = nc.dram_tensor("eb3", [EA, 1], F32, kind="ExternalInput")
    return t


def _load_weights(nc, consts, td, node: bool, edge: bool, with_eb3=True):
    sb = {}
    names = []
    if node:
        names += ["nw1", "nw2", "nw3", "nb1", "nb2", "nb3"]
    if edge:
        names += ["ew1", "ew2", "ew3", "eb1", "eb2"]
        if with_eb3:
            names += ["eb3"]
    for n in names:
        d = td[n]
        sb[n] = consts.tile(list(d.shape), d.dtype, tag=n, name=n)
        nc.sync.dma_start(sb[n], d[:])
    return sb


def _build_fused2():
    """Structured fast path: interleaved node/edge chunks, bf16 output,
    edge bias eb3 added on host.

    Iterations are processed in pairs sharing one L1 psum tile so the h1
    bias+relu runs as a single 1024-col instruction per engine. The PE
    emission order is software-pipelined (next pair's L1 matmuls sit
    between L3(i) and L2(i+1)) so the single-buffered L2 psum never
    stalls the PE. GPSIMD cannot touch PSUM, so it only gets SBUF work:
    the fe broadcast and the first half of the node datapoint reduction.
    """
    nc = bacc.Bacc("TRN2", target_bir_lowering=False)
    td = _declare_weights(nc, with_eb3=False)
    xT_d = nc.dram_tensor("xT", [128, RC], BF16, kind="ExternalInput")
    attrT_d = nc.dram_tensor("attrT", [64, EC], BF16, kind="ExternalInput")
    outT_d = nc.dram_tensor("outT", [64, EC], BF16, kind="ExternalOutput")

    with tile.TileContext(nc) as tc:
        with (
            tc.tile_pool(name="consts", bufs=1) as consts,
            tc.tile_pool(name="xin", bufs=6) as xin,
            tc.tile_pool(name="hid", bufs=2) as hid,
            tc.tile_pool(name="oot", bufs=4) as oot,
            tc.tile_pool(name="hsp", bufs=2) as hsp,
            tc.tile_pool(name="ps1", bufs=1, space="PSUM") as ps1,
            tc.tile_pool(name="ps2", bufs=1, space="PSUM") as ps2,
            tc.tile_pool(name="ps3", bufs=2, space="PSUM") as ps3,
        ):
            w = _load_weights(nc, consts, td, node=True, edge=True, with_eb3=False)
            feT = consts.tile([ODE, GC], BF16, tag="feT")

            # Streams of per-pair work items. Each item is a dict of
            # closures emitted in pipelined order:
            #   rt(pair)   - input tiles ready (dma/bcast), returns tiles
            #   l1(pair)   - L1 matmuls into the shared l1 psum + h1 acts
            #   body(iter) - L2 + h2 acts + L3 + out for one iteration
            #   tail()     - end-of-chunk work (fe matmul) or None

            def node_pairs(c):
                hsum = hsp.tile([128, 2, CH], BF16, tag="hsum", name="hsum")
                for p in range(NNI // 2):
                    def mk_rt(c=c, p=p):
                        xts = []
                        for j in (0, 1):
                            col0 = c * CH * NDATA + (2 * p + j) * TN
                            xt = xin.tile([128, TN], BF16, tag="xt", name="xt")
                            nc.sync.dma_start(xt, xT_d[:, col0:col0 + TN])
                            xts.append(xt)
                        return xts

                    def mk_l1(xts):
                        l1 = ps1.tile([128, 2, 2, TN], F32, tag="l1", name="l1")
                        h1 = hid.tile([128, 2, 2, TN], BF16, tag="h1", name="h1")
                        for j in (0, 1):
                            nc.tensor.matmul(l1[:, 0, j], w["nw1"][:, 0:128],
                                             xts[j], start=True, stop=True)
                            nc.tensor.matmul(l1[:, 1, j], w["nw1"][:, 128:256],
                                             xts[j], start=True, stop=True)
                        nc.scalar.activation(h1[:, 0], l1[:, 0], RELU,
                                             bias=w["nb1"][:, 0:1])
                        nc.vector.tensor_scalar(
                            out=h1[:, 1], in0=l1[:, 1], scalar1=w["nb1"][:, 1:2],
                            scalar2=0.0, op0=ADD, op1=MAX)
                        return h1

                    def mk_body(h1, j, c=c, p=p):
                        i = 2 * p + j
                        l2 = ps2.tile([128, 2, TN], F32, tag="l2", name="l2")
                        for m in (0, 1):
                            for k in (0, 1):
                                nc.tensor.matmul(
                                    l2[:, m], w["nw2"][:, k, m * 128:(m + 1) * 128],
                                    h1[:, k, j], start=(k == 0), stop=(k == 1))
                        h2 = hid.tile([128, 2, TN], BF16, tag="h2", name="h2")
                        nc.scalar.activation(h2[:, 0], l2[:, 0], RELU,
                                             bias=w["nb2"][:, 0:1])
                        if i % 2 == 0:
                            nc.scalar.activation(h2[:, 1], l2[:, 1], RELU,
                                                 bias=w["nb2"][:, 1:2])
                        else:
                            nc.vector.tensor_scalar(
                                out=h2[:, 1], in0=l2[:, 1],
                                scalar1=w["nb2"][:, 1:2],
                                scalar2=0.0, op0=ADD, op1=MAX)
                        # datapoint reduction, stage 1 on GPSIMD (SBUF only):
                        # fold the two 16-datapoint halves, then DVE reduces 16.
                        h2r = h2.rearrange("c k (g d) -> c k g d", d=NDATA)
                        part = hid.tile([128, 2, GPNI, NDATA // 2], BF16,
                                        tag="part", name="part")
                        with nc.allow_low_precision(reason="bf16 partial sums"):
                            nc.gpsimd.tensor_tensor(
                                out=part, in0=h2r[:, :, :, 0:NDATA // 2],
                                in1=h2r[:, :, :, NDATA // 2:NDATA], op=ADD)
                            nc.vector.reduce_sum(
                                out=hsum[:, :, i * GPNI:(i + 1) * GPNI],
                                in_=part.rearrange("c k g d -> c (k g) d"),
                                axis=AXX)

                    def mk_tail(c=c, p=p):
                        if p != NNI // 2 - 1:
                            return
                        l3 = ps3.tile([ODE, CH], F32, tag="l3", name="l3f")
                        for k in (0, 1):
                            nc.tensor.matmul(l3, w["nw3"][:, k], hsum[:, k],
                                             start=(k == 0), stop=(k == 1))
                        nc.scalar.activation(feT[:, c * CH:(c + 1) * CH], l3,
                                             IDENT, bias=w["nb3"])

                    yield dict(rt=mk_rt, l1=mk_l1, body=mk_body, tail=mk_tail)

            def edge_pairs(c):
                for p in range(NEI // 2):
                    def mk_rt(c=c, p=p):
                        rts = []
                        for j in (0, 1):
                            i = 2 * p + j
                            e0 = c * CH * EPG + i * TN
                            g0 = c * CH + i * GPEI
                            rt = xin.tile([128, TN], BF16, tag="rt", name="rt")
                            nc.sync.dma_start(rt[64:128], attrT_d[:, e0:e0 + TN])
                            # fe broadcast: Pool's copy is slow (~3.6ns/col),
                            # so give it 3 graphs and DVE (2x sbuf copy) 1.
                            rtg = rt[0:64].rearrange("c (g e) -> c g e", e=EPG)
                            nc.vector.tensor_copy(
                                out=rtg[:, 0:1],
                                in_=feT[:, g0:g0 + 1, None].to_broadcast(
                                    [ODE, 1, EPG]))
                            nc.gpsimd.tensor_copy(
                                out=rtg[:, 1:GPEI],
                                in_=feT[:, g0 + 1:g0 + GPEI, None].to_broadcast(
                                    [ODE, GPEI - 1, EPG]))
                            rts.append(rt)
                        return rts

                    def mk_l1(rts):
                        l1 = ps1.tile([128, 2, 2, TN], F32, tag="l1", name="l1")
                        h1 = hid.tile([128, 2, 2, TN], BF16, tag="h1", name="h1")
                        for j in (0, 1):
                            nc.tensor.matmul(l1[:, 0, j], w["ew1"][:, 0:128],
                                             rts[j], start=True, stop=True)
                            nc.tensor.matmul(l1[:, 1, j], w["ew1"][:, 128:256],
                                             rts[j], start=True, stop=True)
                        nc.scalar.activation(h1[:, 0], l1[:, 0], RELU,
                                             bias=w["eb1"][:, 0:1])
                        nc.vector.tensor_scalar(
                            out=h1[:, 1], in0=l1[:, 1], scalar1=w["eb1"][:, 1:2],
                            scalar2=0.0, op0=ADD, op1=MAX)
                        return h1

                    def mk_body(h1, j, c=c, p=p):
                        i = 2 * p + j
                        e0 = c * CH * EPG + i * TN
                        l2 = ps2.tile([128, 2, TN], F32, tag="l2", name="l2")
                        for m in (0, 1):
                            for k in (0, 1):
                                nc.tensor.matmul(
                                    l2[:, m], w["ew2"][:, k, m * 128:(m + 1) * 128],
                                    h1[:, k, j], start=(k == 0), stop=(k == 1))
                        h2 = hid.tile([128, 2, TN], BF16, tag="h2", name="h2")
                        nc.scalar.activation(h2[:, 0], l2[:, 0], RELU,
                                             bias=w["eb2"][:, 0:1])
                        nc.vector.tensor_scalar(
                            out=h2[:, 1], in0=l2[:, 1], scalar1=w["eb2"][:, 1:2],
                            scalar2=0.0, op0=ADD, op1=MAX)
                        l3 = ps3.tile([ODE, TN], F32, tag="l3", name="l3")
                        for k in (0, 1):
                            nc.tensor.matmul(l3, w["ew3"][:, k], h2[:, k],
                                             start=(k == 0), stop=(k == 1))
                        ot = oot.tile([ODE, TN], BF16, tag="ot", name="ot")
                        with nc.allow_low_precision(reason="bf16 out, bias on host"):
                            if i % 4 != 3:
                                nc.scalar.activation(
                                    ot, l3, mybir.ActivationFunctionType.Copy)
                            else:
                                nc.vector.tensor_copy(out=ot, in_=l3)
                        nc.sync.dma_start(outT_d[:, e0:e0 + TN], ot)

                    yield dict(rt=mk_rt, l1=mk_l1, body=mk_body, tail=None)

            def run_pipelined(pairs):
                """Emit pair stream software-pipelined: the L1 matmuls of
                pair p+1 are emitted between body(2p) and body(2p+1)."""
                pairs = list(pairs)
                # prologue: inputs for pair 0 and 1, L1 of pair 0
                if not pairs:
                    return
                rt_cur = pairs[0]["rt"]()
                h1_cur = pairs[0]["l1"](rt_cur)
                rt_next = pairs[1]["rt"]() if len(pairs) > 1 else None
                for p, pr in enumerate(pairs):
                    pr["body"](h1_cur, 0)
                    if p + 1 < len(pairs):
                        h1_next = pairs[p + 1]["l1"](rt_next)
                        rt_next = pairs[p + 2]["rt"]() if p + 2 < len(pairs) else None
                    else:
                        h1_next = None
                    pr["body"](h1_cur, 1)
                    if pr["tail"] is not None:
                        pr["tail"]()
                    h1_cur = h1_next

            streams = [list(node_pairs(0))]
            for c in range(NCH):
                if c + 1 < NCH:
                    streams.append(list(node_pairs(c + 1)))
                streams.append(list(edge_pairs(c)))
            run_pipelined([pr for st in streams for pr in st])
    nc.finalize()
    return nc


# ---------------- general fallback (arbitrary edge_index/batch) ----------------

def _emit_node_stage(nc, pools, w, xT_d, hsum):
    consts, xin, hid, ps1, ps2, ps3 = pools
    GT = TN // NDATA
    for p in range(RC // (2 * TN)):
        r0 = p * 2 * TN
        xtp = xin.tile([128, 2, TN], BF16, tag="xt")
        nc.sync.dma_start(xtp, xT_d[:, r0:r0 + 2 * TN].rearrange("c (t e) -> c t e", t=2))
        h1p = hid.tile([128, 2, 2, TN], BF16, tag="h1")
        for t01 in (0, 1):
            ps_a = ps1.tile([128, TN], F32, tag="l1a")
            ps_b = ps1.tile([128, TN], F32, tag="l1b")
            nc.tensor.matmul(ps_a, w["nw1"][:, 0:128], xtp[:, t01], start=True, stop=True)
            nc.tensor.matmul(ps_b, w["nw1"][:, 128:256], xtp[:, t01], start=True, stop=True)
            nc.scalar.activation(h1p[:, 0, t01], ps_a, RELU, bias=w["nb1"][:, 0:1])
            nc.vector.tensor_scalar(
                out=h1p[:, 1, t01], in0=ps_b, scalar1=w["nb1"][:, 1:2], scalar2=0.0,
                op0=ADD, op1=MAX,
            )
        l2ap = ps2.tile([128, 2, TN], F32, tag="l2a")
        l2bp = ps2.tile([128, 2, TN], F32, tag="l2b")
        for t01 in (0, 1):
            for k in (0, 1):
                nc.tensor.matmul(l2ap[:, t01], w["nw2"][:, k, 0:128], h1p[:, k, t01],
                                 start=(k == 0), stop=(k == 1))
            for k in (0, 1):
                nc.tensor.matmul(l2bp[:, t01], w["nw2"][:, k, 128:256], h1p[:, k, t01],
                                 start=(k == 0), stop=(k == 1))
        h2p = hid.tile([128, 2, 2, TN], BF16, tag="h2")
        nc.scalar.activation(h2p[:, 0], l2ap, RELU, bias=w["nb2"][:, 0:1])
        nc.vector.tensor_scalar(
            out=h2p[:, 1], in0=l2bp, scalar1=w["nb2"][:, 1:2], scalar2=0.0,
            op0=ADD, op1=MAX,
        )
        with nc.allow_low_precision(reason="bf16 reduce feeds bf16 matmul"):
            nc.vector.reduce_sum(
                out=hsum[:, :, p * 2 * GT:(p + 1) * 2 * GT],
                in_=h2p.rearrange("c k t (g d) -> c (k t g) d", d=NDATA),
                axis=AXX,
            )
    ps_f = ps3.tile([ODE, 2, TN], F32, tag="l3")
    for k in (0, 1):
        nc.tensor.matmul(ps_f[:, 0], w["nw3"][:, k], hsum[:, k],
                         start=(k == 0), stop=(k == 1))
    return ps_f[:, 0]


def _emit_edge_stage(nc, pools, w, attrT_d, outT_d, feTg_d):
    consts, xin, hid, ps1, ps2, ps3 = pools
    for p in range(EC // (2 * TN)):
        e0 = p * 2 * TN
        rtp = xin.tile([128, 2, TN], BF16, tag="rt")
        nc.sync.dma_start(rtp[64:128],
                          attrT_d[:, e0:e0 + 2 * TN].rearrange("c (t e) -> c t e", t=2))
        nc.sync.dma_start(rtp[0:64],
                          feTg_d[:, e0:e0 + 2 * TN].rearrange("c (t e) -> c t e", t=2))
        e1p = hid.tile([128, 2, 2, TN], BF16, tag="h1")
        for t01 in (0, 1):
            ps_a = ps1.tile([128, TN], F32, tag="l1a")
            ps_b = ps1.tile([128, TN], F32, tag="l1b")
            nc.tensor.matmul(ps_a, w["ew1"][:, 0:128], rtp[:, t01], start=True, stop=True)
            nc.tensor.matmul(ps_b, w["ew1"][:, 128:256], rtp[:, t01], start=True, stop=True)
            nc.scalar.activation(e1p[:, 0, t01], ps_a, RELU, bias=w["eb1"][:, 0:1])
            nc.vector.tensor_scalar(
                out=e1p[:, 1, t01], in0=ps_b, scalar1=w["eb1"][:, 1:2], scalar2=0.0,
                op0=ADD, op1=MAX,
            )
        l2ap = ps2.tile([128, 2, TN], F32, tag="l2a")
        l2bp = ps2.tile([128, 2, TN], F32, tag="l2b")
        for t01 in (0, 1):
            for k in (0, 1):
                nc.tensor.matmul(l2ap[:, t01], w["ew2"][:, k, 0:128], e1p[:, k, t01],
                                 start=(k == 0), stop=(k == 1))
            for k in (0, 1):
                nc.tensor.matmul(l2bp[:, t01], w["ew2"][:, k, 128:256], e1p[:, k, t01],
                                 start=(k == 0), stop=(k == 1))
        e2p = hid.tile([128, 2, 2, TN], BF16, tag="h2")
        nc.scalar.activation(e2p[:, 0], l2ap, RELU, bias=w["eb2"][:, 0:1])
        nc.vector.tensor_scalar(
            out=e2p[:, 1], in0=l2bp, scalar1=w["eb2"][:, 1:2], scalar2=0.0,
            op0=ADD, op1=MAX,
        )
        l3p = ps3.tile([ODE, 2, TN], F32, tag="l3")
        for t01 in (0, 1):
            for k in (0, 1):
                nc.tensor.matmul(l3p[:, t01], w["ew3"][:, k], e2p[:, k, t01],
                                 start=(k == 0), stop=(k == 1))
        otp = hid.tile([ODE, 2, TN], F32, tag="ot")
        if p % 2 == 0:
            nc.scalar.activation(otp, l3p, IDENT, bias=w["eb3"])
        else:
            nc.vector.tensor_scalar(out=otp, in0=l3p, scalar1=w["eb3"],
                                    scalar2=0.0, op0=ADD, op1=mybir.AluOpType.bypass)
        nc.sync.dma_start(outT_d[:, e0:e0 + 2 * TN],
                          otp.rearrange("c t e -> c (t e)"))


def _build_general(mode):
    """mode: 'node' or 'edge' (general fallback path)."""
    nc = bacc.Bacc("TRN2", target_bir_lowering=False)
    td = _declare_weights(nc)
    if mode == "node":
        xT_d = nc.dram_tensor("xT", [128, RC], BF16, kind="ExternalInput")
        feT_out = nc.dram_tensor("feT", [ODE, GC], F32, kind="ExternalOutput")
    else:
        attrT_d = nc.dram_tensor("attrT", [64, EC], BF16, kind="ExternalInput")
        outT_d = nc.dram_tensor("outT", [64, EC], F32, kind="ExternalOutput")
        feTg_d = nc.dram_tensor("feTg", [64, EC], BF16, kind="ExternalInput")

    with tile.TileContext(nc) as tc:
        with (
            tc.tile_pool(name="consts", bufs=1) as consts,
            tc.tile_pool(name="xin", bufs=4) as xin,
            tc.tile_pool(name="hid", bufs=3) as hid,
            tc.tile_pool(name="ps1", bufs=1, space="PSUM") as ps1,
            tc.tile_pool(name="ps2", bufs=1, space="PSUM") as ps2,
            tc.tile_pool(name="ps3", bufs=1, space="PSUM") as ps3,
        ):
            pools = (consts, xin, hid, ps1, ps2, ps3)
            w = _load_weights(nc, consts, td, node=mode == "node", edge=mode == "edge")
            if mode == "node":
                hsum = consts.tile([128, 2, GC], BF16, tag="hsum")
                ps_f = _emit_node_stage(nc, pools, w, xT_d, hsum)
                feT_sb = consts.tile([ODE, GC], F32, tag="feT")
                nc.scalar.activation(feT_sb, ps_f, IDENT, bias=w["nb3"])
                nc.sync.dma_start(feT_out[:], feT_sb)
            else:
                _emit_edge_stage(nc, pools, w, attrT_d, outT_d, feTg_d)
    nc.finalize()
    return nc


def _get_program(mode):
    if mode not in _PROGRAMS:
        if mode == "fused2":
            _PROGRAMS[mode] = _build_fused2()
        else:
            _PROGRAMS[mode] = _build_general(mode)
    return _PROGRAMS[mode]


def _shared_weight_arrays(kw):
    f = np.float32
    c = np.ascontiguousarray
    return {
        "nw1": c(np.asarray(kw["node_w1"], dtype=f).astype(BF16NP)),
        "nw2": c(np.asarray(kw["node_w2"], dtype=f).reshape(2, 128, H).transpose(1, 0, 2).astype(BF16NP)),
        "nw3": c(np.asarray(kw["node_w3"], dtype=f).reshape(2, 128, ODE).transpose(1, 0, 2).astype(BF16NP)),
        "nb1": c(np.asarray(kw["node_b1"], dtype=f).reshape(2, 128).T),
        "nb2": c(np.asarray(kw["node_b2"], dtype=f).reshape(2, 128).T),
        "nb3": c(np.asarray(kw["node_b3"], dtype=f).reshape(ODE, 1)),
        "ew1": c(np.asarray(kw["edge_w1"], dtype=f).astype(BF16NP)),
        "ew2": c(np.asarray(kw["edge_w2"], dtype=f).reshape(2, 128, H).transpose(1, 0, 2).astype(BF16NP)),
        "ew3": c(np.asarray(kw["edge_w3"], dtype=f).reshape(2, 128, ODE).transpose(1, 0, 2).astype(BF16NP)),
        "eb1": c(np.asarray(kw["edge_b1"], dtype=f).reshape(2, 128).T),
        "eb2": c(np.asarray(kw["edge_b2"], dtype=f).reshape(2, 128).T),
        "eb3": c(np.asarray(kw["edge_b3"], dtype=f).reshape(EA, 1)),
    }


def _x_transposed_per_core(x, c):
    xs = np.asarray(x, dtype=np.float32).reshape(G, ODE, 2, NDATA)[c * GC:(c + 1) * GC]
    return np.ascontiguousarray(xs.transpose(1, 2, 0, 3).reshape(128, RC).astype(BF16NP))


def kernel(x, edge_attr, node_w1, node_b1, node_w2, node_b2, node_w3, node_b3,
           edge_w1, edge_b1, edge_w2, edge_b2, edge_w3, edge_b3,
           edge_index, batch):
    global last_results
    kw = dict(x=x, node_w1=node_w1, node_b1=node_b1, node_w2=node_w2,
              node_b2=node_b2, node_w3=node_w3, node_b3=node_b3,
              edge_w1=edge_w1, edge_b1=edge_b1, edge_w2=edge_w2,
              edge_b2=edge_b2, edge_w3=edge_w3, edge_b3=edge_b3)
    trace = os.environ.get("KERNEL_TRACE", "") == "1"
    if trace:
        _install_trace_shim()

    edge_attr = np.asarray(edge_attr, dtype=np.float32)
    ei = np.asarray(edge_index)
    bt = np.asarray(batch)
    g_src = bt[ei[0]]
    g_dst = bt[ei[1]]
    same = g_src == g_dst
    structured = bool((g_src == np.repeat(np.arange(G), EPG)).all())

    shared = _shared_weight_arrays(kw)
    run_kwargs = dict(core_ids=list(range(NCORES)), trace=trace,
                      trace_cores=[0] if trace else None)

    if structured:
        nc = _get_program("fused2")
        eb3_host = np.asarray(kw["edge_b3"], dtype=np.float32).reshape(1, EA)
        in_maps = []
        for c in range(NCORES):
            m = {k: v for k, v in shared.items() if k != "eb3"}
            m["xT"] = _x_transposed_per_core(x, c)
            m["attrT"] = np.ascontiguousarray(edge_attr[c * EC:(c + 1) * EC].T.astype(BF16NP))
            in_maps.append(m)
        res = run_bass_kernel_spmd(nc, in_maps, **run_kwargs)
        last_results = res
        out = np.empty((E, EA), dtype=np.float32)
        for c in range(NCORES):
            out[c * EC:(c + 1) * EC] = res.results[c]["outT"].T.astype(np.float32)
        out += eb3_host
    else:
        # general path: node stage -> host gather of feature_enc -> edge stage
        nc_node = _get_program("node")
        in_maps = []
        for c in range(NCORES):
            m = dict(shared)
            m["xT"] = _x_transposed_per_core(x, c)
            in_maps.append(m)
        res_n = run_bass_kernel_spmd(nc_node, in_maps, **run_kwargs)
        feT_full = np.concatenate([res_n.results[c]["feT"] for c in range(NCORES)],
                                  axis=1)          # [64, G]
        feTg = feT_full[:, g_src]                   # [64, E]
        nc_edge = _get_program("edge")
        in_maps = []
        for c in range(NCORES):
            m = dict(shared)
            m["attrT"] = np.ascontiguousarray(edge_attr[c * EC:(c + 1) * EC].T.astype(BF16NP))
            m["feTg"] = np.ascontiguousarray(feTg[:, c * EC:(c + 1) * EC].astype(BF16NP))
            in_maps.append(m)
        res = run_bass_kernel_spmd(nc_edge, in_maps, **run_kwargs)
        last_results = res
        out = np.empty((E, EA), dtype=np.float32)
        for c in range(NCORES):
            out[c * EC:(c + 1) * EC] = res.results[c]["outT"].T
    if not same.all():
        out = np.where(same[:, None], out, edge_attr)
    return out
